# revision 29
# baseline (speedup 1.0000x reference)
"""nn_FM_49701361549558 — FM embedding lookup on 8 TRN2 NeuronCores.

Sharding: data-parallel over the batch (16384 -> 8 x 2048) with row-sharded
tables: each core's HBM holds exactly the rows its batch shard references
(user/item deduplicated, meta tables shipped whole), fused into one table so
a single per-core dma_gather stream fetches everything.

Gathered row = exactly the 64 bf16 embedding factors: 128 B reads on a
256 B stride via a directly-constructed InstDMAGatherAnt (the bass wrapper
would force 256 B reads, doubling HBM traffic; the ISA itself only
constrains the row *stride* to 256 B multiples).  single_packet=False lets
one gather carry 2048 indices (the single-packet ucode path faults above
1024), so 4 gathers amortize the ~1 us SWDGE descriptor-gen fixed cost
that dominated the 8-gather baseline.  The four f32 linear-term values per
item ride along with the index upload (32 KB vs the 2 MB gather).

Device pipeline per chunk (chunk = t t-slots = 512*t gather indices; batch
item b = p*16 + col lives at partition p, output column col; gather column
order [U(t)|M0(t)|I(t)|M1(t)] so every DVE operand is a contiguous block):
  * Pool: one dma_gather per chunk, then a prepare_only dma_scatter_add
    whose descriptors write the output rows; after the sigmoid one cheap
    trigger_dma fires them (skips the ~1.3 us HWDGE+DGE-delay tail of a
    late hwdge dma_start).
  * DVE, per t-slot, via scalar_tensor_tensor's fused f32 accumulator:
    zz0 = sum(U*I + M0*M1) and (after ab = [U+I | M0+M1], one 2x-mode
    TensorTensor per chunk) zz1 = sum(ab.lo*ab.hi); a 3-element
    TensorReduce over [zz0 | zz1 | linsum] — staggered into the next
    chunk's slot to save a drain — yields the f32 logit.  Per-chunk cost
    is ~1.7 us against a ~1.7 us/chunk gather delivery rate, so the whole
    pipeline runs back-to-back on every engine.
  * ACT: single final Sigmoid (its act-table load is forced into ACT's
    idle window by an early scale=0 dummy activation).
"""

import contextlib

import numpy as np
import ml_dtypes

import concourse.mybir as mybir
from concourse import bacc
from concourse.bass_utils import run_bass_kernel_spmd

P = 128
B = 16384
N_CORES = 8
BL = B // N_CORES          # 2048 per core
T = BL // P                # 16 t-slots
F = 64
NL = 8                     # lin block width per item in aux, in i16 units
NLF = 4                    # = 4 f32 lin values (user, item, meta0, meta1)
EPAD = 128                 # stored table row stride in bf16 elems (= 256 B)
N_USERS = 1_000_000
N_ITEMS = 100_000
N_M0 = 1_000
N_M1 = 20_000

f32 = mybir.dt.float32
i16 = mybir.dt.int16
bf16 = mybir.dt.bfloat16

# t-slots per gather chunk (1 t-slot = 128 items = 512 gather indices).
CHUNKS = (4, 4, 4, 4)
POOL_AB = (2, 3)           # chunks whose [U+I|M0+M1] add runs on idle Pool
GELEM = F                  # gather elem: F (128 B reads) or EPAD (full 256 B)
OUT_SCATTER = True         # output via prepared scatter+trigger vs SP hwdge
BASES = (0, BL, 2 * BL, 2 * BL + N_M0)
N_ROWS = 2 * BL + N_M0 + N_M1                # 25096 (< 2^15)
N_IDX = 4 * BL                               # 8192 gathered rows per core
NIC = N_IDX // 16                            # idx columns (16-part wrap)
OPAD = 64                                    # out row f32 elems (= 256 B)


def dma_gather_raw(eng, out_ap, in_ap, idxs_ap, num_idxs, elem_size,
                   elem_step, queue_num=0, single_packet=True):
    """BassGpSimd.dma_gather (non-transpose, DRAM source, self-triggered)
    minus the elem_size%256B assert — the ISA only requires the source row
    STRIDE (elem_step) to be a 256 B multiple; elem_size is free."""
    assert idxs_ap.dtype == mybir.dt.int16
    assert in_ap.dtype == out_ap.dtype
    stride_bytes = elem_step * mybir.dt.size(in_ap.dtype)
    stride_bytes_256, rem = divmod(stride_bytes, 256)
    assert rem == 0 and stride_bytes_256 < 256
    assert num_idxs % 128 == 0
    assert in_ap.ap[-1][1] == out_ap.ap[-1][1] == elem_size
    assert out_ap.ap[0][1] * out_ap.ap[1][1] == num_idxs
    assert in_ap.ap[0][0] == elem_step
    _in_ap = eng.lower_ap_dma(in_ap, for_custom_bir_dma=True)
    _idxs_ap = eng.lower_ap(idxs_ap)
    _out_ap = eng.lower_ap(out_ap)
    return eng.add_instruction(
        mybir.InstDMAGatherAnt(
            name=eng.bass.get_next_instruction_name(),
            ins=[*_in_ap, _idxs_ap,
                 eng.lower_val_access(eng.to_reg(num_idxs))],
            outs=[_out_ap],
            transpose=False,
            num_idxs=num_idxs,
            elem_size=elem_size,
            stride_bytes_256=stride_bytes_256,
            gen_mode=0,
            single_packet=single_packet,
            queue_num=queue_num,
            sbuf_tokens_per_rank=0,
            sbuf_free_dim_per_rank=0,
            sbuf_free_dim_pad_per_rank=0,
            sbuf_byte_offset=0,
        )
    )


def build_nc(chunks=None, gelem=None, out_scatter=None, pool_ab=None):
    chunks = CHUNKS if chunks is None else chunks
    gelem = GELEM if gelem is None else gelem
    out_scatter = OUT_SCATTER if out_scatter is None else out_scatter
    pool_ab = POOL_AB if pool_ab is None else pool_ab
    assert sum(chunks) == T
    nc = bacc.Bacc(None, target_bir_lowering=False)
    # aux = [idx (512 cols, 16-part wrapped) | out-scatter iota (8 cols) |
    #        lin pairs (T*8 bf16 viewed as i16, all partitions)]
    idx_d = nc.declare_dram_parameter("aux", [P, NIC + 8 + T * NL], i16,
                                      isOutput=False)
    tab_d = nc.declare_dram_parameter("table", [N_ROWS, EPAD], bf16,
                                      isOutput=False)
    out_d = nc.declare_dram_parameter("out", [P, OPAD], f32, isOutput=True)

    C = len(chunks)
    with contextlib.ExitStack() as ctx:
        aux_sb = ctx.enter_context(
            nc.sbuf_tensor("aux_sb", [P, NIC + 8], i16))
        gs = [ctx.enter_context(nc.sbuf_tensor(f"g{k}", [P, 4 * t, gelem], bf16))
              for k, t in enumerate(chunks)]
        abs_ = [ctx.enter_context(nc.sbuf_tensor(f"ab{k}", [P, 2 * t, F], bf16))
                for k, t in enumerate(chunks)]
        dmyA = ctx.enter_context(nc.sbuf_tensor("dmyA", [P, T, 2, F], bf16))
        dmyB = ctx.enter_context(nc.sbuf_tensor("dmyB", [P, T, F], bf16))
        linb = ctx.enter_context(nc.sbuf_tensor("linb", [P, T * NL], i16))
        zz = ctx.enter_context(nc.sbuf_tensor("zz", [P, 3, T], f32))
        z = ctx.enter_context(nc.sbuf_tensor("z", [P, T], f32))
        sig = ctx.enter_context(nc.sbuf_tensor("sig", [P, T], f32))
        sgd = ctx.enter_context(nc.sbuf_tensor("sgd", [P, 1], f32))
        isem = ctx.enter_context(nc.semaphore("isem"))
        isem2 = ctx.enter_context(nc.semaphore("isem2"))
        lsem = ctx.enter_context(nc.semaphore("lsem"))
        gsems = [ctx.enter_context(nc.semaphore(f"gsem{k}")) for k in range(C)]
        psem = ctx.enter_context(nc.semaphore("psem"))    # out-desc prep done
        vsem = ctx.enter_context(nc.semaphore("vsem"))    # DVE chunk done
        absem = ctx.enter_context(nc.semaphore("absem"))  # Pool ab blocks
        ssem = ctx.enter_context(nc.semaphore("ssem"))    # sigmoid done
        osem = ctx.enter_context(nc.semaphore("osem"))    # out DMA done
        block = ctx.enter_context(nc.Block())

        @block.gpsimd
        def _(gpsimd):
            c0 = 0
            for k, t in enumerate(chunks):
                gpsimd.wait_ge(isem if k == 0 else isem2, 16)
                n_k = 512 * t
                dma_gather_raw(
                    gpsimd,
                    out_ap=gs[k][:],
                    in_ap=tab_d[:, 0:gelem],
                    idxs_ap=aux_sb[:, c0:c0 + n_k // 16],
                    num_idxs=n_k,
                    elem_size=gelem,
                    elem_step=EPAD,
                    single_packet=False,
                ).then_inc(gsems[k], 16)
                c0 += n_k // 16
            if out_scatter:
                # Pre-generate output scatter descriptors; trigger after sigmoid.
                gpsimd.dma_scatter_add(
                    out_ap=out_d[:, 0:T],
                    in_ap=sig[:].rearrange("p (o t) -> p o t", o=1),
                    idxs_ap=aux_sb[:, NIC:NIC + 8],
                    num_idxs=P,
                    num_idxs_reg=P,
                    elem_size=T,
                    elem_step=OPAD,
                    prepare_only=True,
                    sem=osem,
                ).then_inc(psem, 1)
                for k in pool_ab:
                    gpsimd.wait_ge(gsems[k], 16)
                    g = gs[k][:].rearrange("p (f t) e -> p f t e", f=4)
                    ab = abs_[k][:].rearrange("p (h t) e -> p h t e", h=2)
                    gpsimd.tensor_add(
                        out=ab, in0=g[:, 0:2], in1=g[:, 2:4]).then_inc(
                            absem, 1)
                gpsimd.wait_ge(psem, 1)
                gpsimd.wait_ge(ssem, 1)
                gpsimd.trigger_dma(count=1)
                gpsimd.wait_ge(osem, 16)

        @block.vector
        def _(vector):
            # zz rows: [0] = sum(U*I)+sum(M0*M1), [1] = sum((U+I)*(M0+M1)),
            # [2] = lin sum; z = column-sum of the three (70-cycle reduce).
            vector.wait_ge(lsem, 16)
            vector.tensor_reduce(
                out=zz[:, 2, :], in_=linb[:].bitcast(f32).rearrange(
                    "p (t l) -> p t l", l=NLF),
                axis=mybir.AxisListType.X, op=mybir.AluOpType.add)
            vector.drain()
            cols = []
            col = 0
            for t in chunks:
                cols.append((col, t))
                col += t
            for k, t in enumerate(chunks):
                c0 = cols[k][0]
                vector.wait_ge(gsems[k], 16)
                g = gs[k][:].rearrange("p (f t) e -> p f t e", f=4)
                ab = abs_[k][:].rearrange("p (h t) e -> p h t e", h=2)
                # G1: per-slot fused mul+accum (f32) over the (U,I),(M0,M1)
                # pairs; the ab add; and the PREVIOUS chunk's 3-way combine.
                for tt in range(t):
                    vector.scalar_tensor_tensor(
                        out=dmyA[:, c0 + tt], in0=g[:, 0:2, tt, :],
                        scalar=0.0, in1=g[:, 2:4, tt, :],
                        op0=mybir.AluOpType.add, op1=mybir.AluOpType.mult,
                        accum_out=zz[:, 0, c0 + tt:c0 + tt + 1])
                if k not in pool_ab:
                    vector.tensor_add(
                        out=ab, in0=g[:, 0:2], in1=g[:, 2:4])  # [U+I|M0+M1]
                if k > 0:
                    p0, pt = cols[k - 1]
                    vector.tensor_reduce(
                        out=z[:, p0:p0 + pt],
                        in_=zz[:].rearrange("p r t -> p t r")[:, p0:p0 + pt, :],
                        axis=mybir.AxisListType.X, op=mybir.AluOpType.add,
                    ).then_inc(vsem, 1)
                vector.drain()          # DVE same-engine RAW needs a drain
                if k in pool_ab:
                    vector.wait_ge(absem, pool_ab.index(k) + 1)
                for tt in range(t):
                    vector.scalar_tensor_tensor(
                        out=dmyB[:, c0 + tt], in0=ab[:, 0, tt, :],
                        scalar=0.0, in1=ab[:, 1, tt, :],
                        op0=mybir.AluOpType.add, op1=mybir.AluOpType.mult,
                        accum_out=zz[:, 1, c0 + tt:c0 + tt + 1])
                vector.drain()
            p0, pt = cols[-1]
            vector.tensor_reduce(
                out=z[:, p0:p0 + pt],
                in_=zz[:].rearrange("p r t -> p t r")[:, p0:p0 + pt, :],
                axis=mybir.AxisListType.X, op=mybir.AluOpType.add,
            ).then_inc(vsem, 1)

        @block.scalar
        def _(scalar):
            # scale=0 dummy: forces the sigmoid act-table load into ACT's
            # idle window at t~0.7us instead of the critical tail (input is
            # never read, output overwritten by the real sigmoid below)
            scalar.activation(
                out=sgd[:], in_=sgd[:],
                func=mybir.ActivationFunctionType.Sigmoid, scale=0.0,
            )
            scalar.wait_ge(vsem, C)
            scalar.activation(
                out=sig[:], in_=z[:],
                func=mybir.ActivationFunctionType.Sigmoid,
            ).then_inc(ssem, 1)

        @block.sync
        def _(sync):
            nic0 = 32 * chunks[0]
            sync.dma_start(out=aux_sb[:, 0:nic0],
                           in_=idx_d[:, 0:nic0]).then_inc(isem, 16)
            sync.dma_start(out=aux_sb[:, nic0:],
                           in_=idx_d[:, nic0:NIC + 8]).then_inc(isem2, 16)
            sync.dma_start(out=linb[:], in_=idx_d[:, NIC + 8:]).then_inc(
                lsem, 16)
            if not out_scatter:
                sync.wait_ge(ssem, 1)
                sync.dma_start(out=out_d[:, 0:T], in_=sig[:]).then_inc(osem, 16)
                sync.wait_ge(osem, 16)

    nc.finalize()
    return nc


def host_prepare(inputs, chunks=None):
    """Row-shard the tables per core and build device aux/table tensors."""
    chunks = CHUNKS if chunks is None else chunks
    user_emb = np.asarray(inputs["user_emb"], np.float32)
    item_emb = np.asarray(inputs["item_emb"], np.float32)
    lins = [np.asarray(inputs[n], np.float32).reshape(-1)
            for n in ("user_lin", "item_lin", "meta_lin0", "meta_lin1")]

    uids = np.asarray(inputs["user_ids"]).astype(np.int64)
    iids = np.asarray(inputs["item_ids"]).astype(np.int64)
    meta = np.asarray(inputs["metadata_ids"]).astype(np.int64)

    bf = ml_dtypes.bfloat16
    meta_block = np.zeros((N_M0 + N_M1, EPAD), bf)
    meta_block[:N_M0, :F] = np.asarray(inputs["meta_emb0"], np.float32)
    meta_block[N_M0:, :F] = np.asarray(inputs["meta_emb1"], np.float32)

    per_core_aux, per_core_tab = [], []
    for c in range(N_CORES):
        sl = slice(c * BL, (c + 1) * BL)
        u_uniq, u_inv = np.unique(uids[sl], return_inverse=True)
        i_uniq, i_inv = np.unique(iids[sl], return_inverse=True)

        tab = np.zeros((N_ROWS, EPAD), bf)
        tab[:len(u_uniq), :F] = user_emb[u_uniq]
        tab[BL:BL + len(i_uniq), :F] = item_emb[i_uniq]
        tab[2 * BL:] = meta_block

        # local row index per field, [4, P, T] (batch item b = p*16 + col)
        loc = np.empty((4, P, T), np.int16)
        loc[0] = (u_inv + BASES[0]).reshape(P, T)
        loc[1] = (i_inv + BASES[1]).reshape(P, T)
        loc[2] = (meta[sl, 0] + BASES[2]).reshape(P, T)
        loc[3] = (meta[sl, 1] + BASES[3]).reshape(P, T)

        # gather order per chunk: j = col*128 + p; cols = [U(t)|M0(t)|I(t)|M1(t)]
        blocks = []
        t0 = 0
        for t in chunks:
            fslice = loc[[0, 2, 1, 3], :, t0:t0 + t]        # [U, M0, I, M1]
            u_k = np.ascontiguousarray(
                fslice.transpose(0, 2, 1)                    # [f, tt, p]
            ).reshape(-1)                                    # j = (f*t+tt)*128+p
            blocks.append(u_k.reshape(-1, 16).T)             # [16, n_k/16]
            t0 += t
        idx16 = np.concatenate(blocks, axis=1)               # [16, N_IDX/16]

        # out-scatter iota indices (row p scatters to out row p), 16-part wrap
        oidx = np.arange(P, dtype=np.int16).reshape(-1, 16).T  # [16, 8]

        # lin values [P, T, 4] f32: [user, item, meta0, meta1]
        lp = np.empty((BL, NLF), np.float32)
        ids4 = (uids[sl], iids[sl], meta[sl, 0], meta[sl, 1])
        for f in range(4):
            lp[:, f] = lins[f][ids4[f]]
        lin_i16 = np.ascontiguousarray(
            lp.reshape(P, T * NLF)).view(np.int16)           # [P, T*8]

        aux = np.zeros((P, NIC + 8 + T * NL), np.int16)
        # idx + scatter-iota are 16-partition-wrapped and must be REPLICATED
        # to all 128 partitions: the gather ucode's 8 Q7 cores each read
        # their own 16-partition copy
        aux[:, :NIC] = np.tile(idx16, (P // 16, 1))
        aux[:, NIC:NIC + 8] = np.tile(oidx, (P // 16, 1))
        aux[:, NIC + 8:] = lin_i16
        per_core_aux.append(aux)
        per_core_tab.append(tab)
    return per_core_aux, per_core_tab


_NC_CACHE = None


def _get_nc():
    global _NC_CACHE
    if _NC_CACHE is None:
        _NC_CACHE = build_nc()
    return _NC_CACHE


def kernel(**inputs) -> np.ndarray:
    nc = _get_nc()
    per_core_aux, per_core_tab = host_prepare(inputs)
    in_maps = [
        {"aux": per_core_aux[c], "table": per_core_tab[c]}
        for c in range(N_CORES)
    ]
    res = run_bass_kernel_spmd(nc, in_maps, list(range(N_CORES)))
    return np.concatenate(
        [res.results[c]["out"][:, :T].reshape(-1) for c in range(N_CORES)]
    ).astype(np.float32)


# revision 30
# speedup vs baseline: 1.0040x; 1.0040x over previous
"""nn_FM_49701361549558 — FM embedding lookup on 8 TRN2 NeuronCores.

Sharding: data-parallel over the batch (16384 -> 8 x 2048) with row-sharded
tables: each core's HBM holds exactly the rows its batch shard references
(user/item deduplicated, meta tables shipped whole), fused into one table so
a single per-core dma_gather stream fetches everything.

Gathered row = exactly the 64 bf16 embedding factors: 128 B reads on a
256 B stride via a directly-constructed InstDMAGatherAnt (the bass wrapper
would force 256 B reads, doubling HBM traffic; the ISA itself only
constrains the row *stride* to 256 B multiples).  single_packet=False lets
one gather carry 2048 indices (the single-packet ucode path faults above
1024), so 4 gathers amortize the ~1 us SWDGE descriptor-gen fixed cost
that dominated the 8-gather baseline.  The four f32 linear-term values per
item ride along with the index upload (32 KB vs the 2 MB gather).

Device pipeline per chunk (chunk = t t-slots = 512*t gather indices; batch
item b = p*16 + col lives at partition p, output column col; gather column
order [U(t)|M0(t)|I(t)|M1(t)] so every DVE operand is a contiguous block):
  * Pool: one dma_gather per chunk, then a prepare_only dma_scatter_add
    whose descriptors write the output rows; after the sigmoid one cheap
    trigger_dma fires them (skips the ~1.3 us HWDGE+DGE-delay tail of a
    late hwdge dma_start).
  * DVE, per t-slot, via scalar_tensor_tensor's fused f32 accumulator:
    zz0 = sum(U*I + M0*M1) and (after ab = [U+I | M0+M1], one 2x-mode
    TensorTensor per chunk) zz1 = sum(ab.lo*ab.hi); a 3-element
    TensorReduce over [zz0 | zz1 | linsum] — staggered into the next
    chunk's slot to save a drain — yields the f32 logit.  Per-chunk cost
    is ~1.7 us against a ~1.7 us/chunk gather delivery rate, so the whole
    pipeline runs back-to-back on every engine.
  * ACT: single final Sigmoid (its act-table load is forced into ACT's
    idle window by an early scale=0 dummy activation).
"""

import contextlib

import numpy as np
import ml_dtypes

import concourse.mybir as mybir
from concourse import bacc
from concourse.bass_utils import run_bass_kernel_spmd

P = 128
B = 16384
N_CORES = 8
BL = B // N_CORES          # 2048 per core
T = BL // P                # 16 t-slots
F = 64
NL = 8                     # lin block width per item in aux, in i16 units
NLF = 4                    # = 4 f32 lin values (user, item, meta0, meta1)
EPAD = 128                 # stored table row stride in bf16 elems (= 256 B)
N_USERS = 1_000_000
N_ITEMS = 100_000
N_M0 = 1_000
N_M1 = 20_000

f32 = mybir.dt.float32
i16 = mybir.dt.int16
bf16 = mybir.dt.bfloat16

# t-slots per gather chunk (1 t-slot = 128 items = 512 gather indices).
CHUNKS = (4, 4, 4, 4)
POOL_AB = (2, 3)           # chunks whose [U+I|M0+M1] add runs on idle Pool
AB3_DVE = 1                # trailing ab slots of the LAST chunk kept on DVE
GELEM = F                  # gather elem: F (128 B reads) or EPAD (full 256 B)
OUT_SCATTER = True         # output via prepared scatter+trigger vs SP hwdge
BASES = (0, BL, 2 * BL, 2 * BL + N_M0)
N_ROWS = 2 * BL + N_M0 + N_M1                # 25096 (< 2^15)
N_IDX = 4 * BL                               # 8192 gathered rows per core
NIC = N_IDX // 16                            # idx columns (16-part wrap)
OPAD = 64                                    # out row f32 elems (= 256 B)


def dma_gather_raw(eng, out_ap, in_ap, idxs_ap, num_idxs, elem_size,
                   elem_step, queue_num=0, single_packet=True):
    """BassGpSimd.dma_gather (non-transpose, DRAM source, self-triggered)
    minus the elem_size%256B assert — the ISA only requires the source row
    STRIDE (elem_step) to be a 256 B multiple; elem_size is free."""
    assert idxs_ap.dtype == mybir.dt.int16
    assert in_ap.dtype == out_ap.dtype
    stride_bytes = elem_step * mybir.dt.size(in_ap.dtype)
    stride_bytes_256, rem = divmod(stride_bytes, 256)
    assert rem == 0 and stride_bytes_256 < 256
    assert num_idxs % 128 == 0
    assert in_ap.ap[-1][1] == out_ap.ap[-1][1] == elem_size
    assert out_ap.ap[0][1] * out_ap.ap[1][1] == num_idxs
    assert in_ap.ap[0][0] == elem_step
    _in_ap = eng.lower_ap_dma(in_ap, for_custom_bir_dma=True)
    _idxs_ap = eng.lower_ap(idxs_ap)
    _out_ap = eng.lower_ap(out_ap)
    return eng.add_instruction(
        mybir.InstDMAGatherAnt(
            name=eng.bass.get_next_instruction_name(),
            ins=[*_in_ap, _idxs_ap,
                 eng.lower_val_access(eng.to_reg(num_idxs))],
            outs=[_out_ap],
            transpose=False,
            num_idxs=num_idxs,
            elem_size=elem_size,
            stride_bytes_256=stride_bytes_256,
            gen_mode=0,
            single_packet=single_packet,
            queue_num=queue_num,
            sbuf_tokens_per_rank=0,
            sbuf_free_dim_per_rank=0,
            sbuf_free_dim_pad_per_rank=0,
            sbuf_byte_offset=0,
        )
    )


def build_nc(chunks=None, gelem=None, out_scatter=None, pool_ab=None):
    chunks = CHUNKS if chunks is None else chunks
    gelem = GELEM if gelem is None else gelem
    out_scatter = OUT_SCATTER if out_scatter is None else out_scatter
    pool_ab = POOL_AB if pool_ab is None else pool_ab
    assert sum(chunks) == T
    nc = bacc.Bacc(None, target_bir_lowering=False)
    # aux = [idx (512 cols, 16-part wrapped) | out-scatter iota (8 cols) |
    #        lin pairs (T*8 bf16 viewed as i16, all partitions)]
    idx_d = nc.declare_dram_parameter("aux", [P, NIC + 8 + T * NL], i16,
                                      isOutput=False)
    tab_d = nc.declare_dram_parameter("table", [N_ROWS, EPAD], bf16,
                                      isOutput=False)
    out_d = nc.declare_dram_parameter("out", [P, OPAD], f32, isOutput=True)

    C = len(chunks)
    with contextlib.ExitStack() as ctx:
        aux_sb = ctx.enter_context(
            nc.sbuf_tensor("aux_sb", [P, NIC + 8], i16))
        gs = [ctx.enter_context(nc.sbuf_tensor(f"g{k}", [P, 4 * t, gelem], bf16))
              for k, t in enumerate(chunks)]
        abs_ = [ctx.enter_context(nc.sbuf_tensor(f"ab{k}", [P, 2 * t, F], bf16))
                for k, t in enumerate(chunks)]
        dmyA = ctx.enter_context(nc.sbuf_tensor("dmyA", [P, T, 2, F], bf16))
        dmyB = ctx.enter_context(nc.sbuf_tensor("dmyB", [P, T, F], bf16))
        linb = ctx.enter_context(nc.sbuf_tensor("linb", [P, T * NL], i16))
        zz = ctx.enter_context(nc.sbuf_tensor("zz", [P, 3, T], f32))
        z = ctx.enter_context(nc.sbuf_tensor("z", [P, T], f32))
        sig = ctx.enter_context(nc.sbuf_tensor("sig", [P, T], f32))
        sgd = ctx.enter_context(nc.sbuf_tensor("sgd", [P, 1], f32))
        isem = ctx.enter_context(nc.semaphore("isem"))
        isem2 = ctx.enter_context(nc.semaphore("isem2"))
        lsem = ctx.enter_context(nc.semaphore("lsem"))
        gsems = [ctx.enter_context(nc.semaphore(f"gsem{k}")) for k in range(C)]
        psem = ctx.enter_context(nc.semaphore("psem"))    # out-desc prep done
        vsem = ctx.enter_context(nc.semaphore("vsem"))    # DVE chunk done
        absem = ctx.enter_context(nc.semaphore("absem"))  # Pool ab blocks
        ssem = ctx.enter_context(nc.semaphore("ssem"))    # sigmoid done
        osem = ctx.enter_context(nc.semaphore("osem"))    # out DMA done
        block = ctx.enter_context(nc.Block())

        @block.gpsimd
        def _(gpsimd):
            c0 = 0
            for k, t in enumerate(chunks):
                gpsimd.wait_ge(isem if k == 0 else isem2, 16)
                n_k = 512 * t
                dma_gather_raw(
                    gpsimd,
                    out_ap=gs[k][:],
                    in_ap=tab_d[:, 0:gelem],
                    idxs_ap=aux_sb[:, c0:c0 + n_k // 16],
                    num_idxs=n_k,
                    elem_size=gelem,
                    elem_step=EPAD,
                    single_packet=False,
                ).then_inc(gsems[k], 16)
                c0 += n_k // 16
            if out_scatter:
                # Pre-generate output scatter descriptors; trigger after sigmoid.
                gpsimd.dma_scatter_add(
                    out_ap=out_d[:, 0:T],
                    in_ap=sig[:].rearrange("p (o t) -> p o t", o=1),
                    idxs_ap=aux_sb[:, NIC:NIC + 8],
                    num_idxs=P,
                    num_idxs_reg=P,
                    elem_size=T,
                    elem_step=OPAD,
                    prepare_only=True,
                    sem=osem,
                ).then_inc(psem, 1)
                for k in pool_ab:
                    t = chunks[k]
                    gpsimd.wait_ge(gsems[k], 16)
                    g = gs[k][:].rearrange("p (f t) e -> p f t e", f=4)
                    ab = abs_[k][:].rearrange("p (h t) e -> p h t e", h=2)
                    tp = t - AB3_DVE if k == C - 1 else t
                    gpsimd.tensor_add(
                        out=ab[:, :, 0:tp, :], in0=g[:, 0:2, 0:tp, :],
                        in1=g[:, 2:4, 0:tp, :]).then_inc(absem, 1)
                gpsimd.wait_ge(psem, 1)
                gpsimd.wait_ge(ssem, 1)
                gpsimd.trigger_dma(count=1)
                gpsimd.wait_ge(osem, 16)

        @block.vector
        def _(vector):
            # zz rows: [0] = sum(U*I)+sum(M0*M1), [1] = sum((U+I)*(M0+M1)),
            # [2] = lin sum; z = column-sum of the three (70-cycle reduce).
            vector.wait_ge(lsem, 16)
            vector.tensor_reduce(
                out=zz[:, 2, :], in_=linb[:].bitcast(f32).rearrange(
                    "p (t l) -> p t l", l=NLF),
                axis=mybir.AxisListType.X, op=mybir.AluOpType.add)
            vector.drain()
            cols = []
            col = 0
            for t in chunks:
                cols.append((col, t))
                col += t
            for k, t in enumerate(chunks):
                c0 = cols[k][0]
                vector.wait_ge(gsems[k], 16)
                g = gs[k][:].rearrange("p (f t) e -> p f t e", f=4)
                ab = abs_[k][:].rearrange("p (h t) e -> p h t e", h=2)
                # G1: per-slot fused mul+accum (f32) over the (U,I),(M0,M1)
                # pairs; the ab add; and the PREVIOUS chunk's 3-way combine.
                for tt in range(t):
                    vector.scalar_tensor_tensor(
                        out=dmyA[:, c0 + tt], in0=g[:, 0:2, tt, :],
                        scalar=0.0, in1=g[:, 2:4, tt, :],
                        op0=mybir.AluOpType.add, op1=mybir.AluOpType.mult,
                        accum_out=zz[:, 0, c0 + tt:c0 + tt + 1])
                if k not in pool_ab:
                    vector.tensor_add(
                        out=ab, in0=g[:, 0:2], in1=g[:, 2:4])  # [U+I|M0+M1]
                elif k == C - 1 and AB3_DVE:
                    s0 = t - AB3_DVE
                    vector.tensor_add(
                        out=ab[:, :, s0:, :], in0=g[:, 0:2, s0:, :],
                        in1=g[:, 2:4, s0:, :])
                if k > 0:
                    p0, pt = cols[k - 1]
                    vector.tensor_reduce(
                        out=z[:, p0:p0 + pt],
                        in_=zz[:].rearrange("p r t -> p t r")[:, p0:p0 + pt, :],
                        axis=mybir.AxisListType.X, op=mybir.AluOpType.add,
                    ).then_inc(vsem, 1)
                vector.drain()          # DVE same-engine RAW needs a drain
                if k in pool_ab:
                    vector.wait_ge(absem, pool_ab.index(k) + 1)
                for tt in range(t):
                    vector.scalar_tensor_tensor(
                        out=dmyB[:, c0 + tt], in0=ab[:, 0, tt, :],
                        scalar=0.0, in1=ab[:, 1, tt, :],
                        op0=mybir.AluOpType.add, op1=mybir.AluOpType.mult,
                        accum_out=zz[:, 1, c0 + tt:c0 + tt + 1])
                vector.drain()
            p0, pt = cols[-1]
            vector.tensor_reduce(
                out=z[:, p0:p0 + pt],
                in_=zz[:].rearrange("p r t -> p t r")[:, p0:p0 + pt, :],
                axis=mybir.AxisListType.X, op=mybir.AluOpType.add,
            ).then_inc(vsem, 1)

        @block.scalar
        def _(scalar):
            # scale=0 dummy: forces the sigmoid act-table load into ACT's
            # idle window at t~0.7us instead of the critical tail (input is
            # never read, output overwritten by the real sigmoid below)
            scalar.activation(
                out=sgd[:], in_=sgd[:],
                func=mybir.ActivationFunctionType.Sigmoid, scale=0.0,
            )
            scalar.wait_ge(vsem, C)
            scalar.activation(
                out=sig[:], in_=z[:],
                func=mybir.ActivationFunctionType.Sigmoid,
            ).then_inc(ssem, 1)

        @block.sync
        def _(sync):
            nic0 = 32 * chunks[0]
            sync.dma_start(out=aux_sb[:, 0:nic0],
                           in_=idx_d[:, 0:nic0]).then_inc(isem, 16)
            sync.dma_start(out=aux_sb[:, nic0:],
                           in_=idx_d[:, nic0:NIC + 8]).then_inc(isem2, 16)
            sync.dma_start(out=linb[:], in_=idx_d[:, NIC + 8:]).then_inc(
                lsem, 16)
            if not out_scatter:
                sync.wait_ge(ssem, 1)
                sync.dma_start(out=out_d[:, 0:T], in_=sig[:]).then_inc(osem, 16)
                sync.wait_ge(osem, 16)

    nc.finalize()
    return nc


def host_prepare(inputs, chunks=None):
    """Row-shard the tables per core and build device aux/table tensors."""
    chunks = CHUNKS if chunks is None else chunks
    user_emb = np.asarray(inputs["user_emb"], np.float32)
    item_emb = np.asarray(inputs["item_emb"], np.float32)
    lins = [np.asarray(inputs[n], np.float32).reshape(-1)
            for n in ("user_lin", "item_lin", "meta_lin0", "meta_lin1")]

    uids = np.asarray(inputs["user_ids"]).astype(np.int64)
    iids = np.asarray(inputs["item_ids"]).astype(np.int64)
    meta = np.asarray(inputs["metadata_ids"]).astype(np.int64)

    bf = ml_dtypes.bfloat16
    meta_block = np.zeros((N_M0 + N_M1, EPAD), bf)
    meta_block[:N_M0, :F] = np.asarray(inputs["meta_emb0"], np.float32)
    meta_block[N_M0:, :F] = np.asarray(inputs["meta_emb1"], np.float32)

    per_core_aux, per_core_tab = [], []
    for c in range(N_CORES):
        sl = slice(c * BL, (c + 1) * BL)
        u_uniq, u_inv = np.unique(uids[sl], return_inverse=True)
        i_uniq, i_inv = np.unique(iids[sl], return_inverse=True)

        tab = np.zeros((N_ROWS, EPAD), bf)
        tab[:len(u_uniq), :F] = user_emb[u_uniq]
        tab[BL:BL + len(i_uniq), :F] = item_emb[i_uniq]
        tab[2 * BL:] = meta_block

        # local row index per field, [4, P, T] (batch item b = p*16 + col)
        loc = np.empty((4, P, T), np.int16)
        loc[0] = (u_inv + BASES[0]).reshape(P, T)
        loc[1] = (i_inv + BASES[1]).reshape(P, T)
        loc[2] = (meta[sl, 0] + BASES[2]).reshape(P, T)
        loc[3] = (meta[sl, 1] + BASES[3]).reshape(P, T)

        # gather order per chunk: j = col*128 + p; cols = [U(t)|M0(t)|I(t)|M1(t)]
        blocks = []
        t0 = 0
        for t in chunks:
            fslice = loc[[0, 2, 1, 3], :, t0:t0 + t]        # [U, M0, I, M1]
            u_k = np.ascontiguousarray(
                fslice.transpose(0, 2, 1)                    # [f, tt, p]
            ).reshape(-1)                                    # j = (f*t+tt)*128+p
            blocks.append(u_k.reshape(-1, 16).T)             # [16, n_k/16]
            t0 += t
        idx16 = np.concatenate(blocks, axis=1)               # [16, N_IDX/16]

        # out-scatter iota indices (row p scatters to out row p), 16-part wrap
        oidx = np.arange(P, dtype=np.int16).reshape(-1, 16).T  # [16, 8]

        # lin values [P, T, 4] f32: [user, item, meta0, meta1]
        lp = np.empty((BL, NLF), np.float32)
        ids4 = (uids[sl], iids[sl], meta[sl, 0], meta[sl, 1])
        for f in range(4):
            lp[:, f] = lins[f][ids4[f]]
        lin_i16 = np.ascontiguousarray(
            lp.reshape(P, T * NLF)).view(np.int16)           # [P, T*8]

        aux = np.zeros((P, NIC + 8 + T * NL), np.int16)
        # idx + scatter-iota are 16-partition-wrapped and must be REPLICATED
        # to all 128 partitions: the gather ucode's 8 Q7 cores each read
        # their own 16-partition copy
        aux[:, :NIC] = np.tile(idx16, (P // 16, 1))
        aux[:, NIC:NIC + 8] = np.tile(oidx, (P // 16, 1))
        aux[:, NIC + 8:] = lin_i16
        per_core_aux.append(aux)
        per_core_tab.append(tab)
    return per_core_aux, per_core_tab


_NC_CACHE = None


def _get_nc():
    global _NC_CACHE
    if _NC_CACHE is None:
        _NC_CACHE = build_nc()
    return _NC_CACHE


def kernel(**inputs) -> np.ndarray:
    nc = _get_nc()
    per_core_aux, per_core_tab = host_prepare(inputs)
    in_maps = [
        {"aux": per_core_aux[c], "table": per_core_tab[c]}
        for c in range(N_CORES)
    ]
    res = run_bass_kernel_spmd(nc, in_maps, list(range(N_CORES)))
    return np.concatenate(
        [res.results[c]["out"][:, :T].reshape(-1) for c in range(N_CORES)]
    ).astype(np.float32)
